# revision 34
# baseline (speedup 1.0000x reference)
"""HGAT layer Trainium2 Bass kernel (v7).

Math (per batch element b, per group pair):
  q,k,v = relu(x @ w + b) for each group
  4 masked attentions (00, 11, 01, 10), each NH=4 heads of H=32
  inner/inter = relu(attn @ wo + bo); out_g = concat(inner_g, inter_g) @ wf_g + bf_g

Design notes (v6/v7/v8, measured on HW via (T129-T65)/64 differential timing;
v5 baseline 239us -> v7 ~150us -> v8 (cross-iteration pipeline) 139us/iter):
  - PE emission is grouped for tile_position concurrency: per (pair, chunk)
    the 4 score matmuls (row groups 32h) are adjacent, then the 4 av and
    4 den matmuls (col groups 32h) are adjacent.  HW runs same-kind packed
    matmuls concurrently (~3x for 4-way packing); interleaving them with
    full-array matmuls serializes the array.
  - av/den accumulation open/close matmuls removed: start lands on the c==0
    matmul and stop on the c==3 matmul of each 32-row head slice
    (skip_group_check works around a CoreSim bookkeeping bug for slices at
    nonzero partition offsets).
  - All ACT functions resolve to one HW table (natural_log_exp_and_others)
    by narrowing the cached act-table dict the load-insertion pass reads;
    the greedy picker otherwise thrashes exp_and_others <-> natural_log
    (1.3us per reload, 32 reloads per iteration).
  - softmax exp split per score tile between three engine paths to balance
    DVE/ACT/GPSIMD (each lands ~105us/core busy):
      B: DVE scalar_tensor_tensor Schraudolph i16=(sc+CADD)*(M*mask) (1.2us)
      A: ACT exp (~1.0us) + DVE f16 mask multiply (0.6us, 2x mode)
      C: ACT exp + GPSIMD mask multiply (~2.1us, fills the idle Pool engine)
    Mask values are stored as M=261.25 so one buffer serves all paths.
  - 1/den via the single-instruction custom-DVE reciprocal_approx_fast
    (~51 ULP; den is 100..1500 so no edge cases); the an=av*rden multiply
    sits on the same DVE queue so the tail has no ACT round-trip.
  - Tails are emitted in delayed phases (TAIL_DELAY/TAIL2/OUT_DELAY pops
    behind the av matmuls): engine queues are strict FIFO, so a
    cross-engine-dependent op parked at a queue head blocks everything
    behind it.
  - Dead ends (tried, rejected): fp8 DoubleRow av/den (walrus forbids
    DoubleRow + tile_position col offsets), DMA accumulate-multiply masks
    (walrus compile fails), additive mask matmuls (PE cost exceeds the
    elementwise saving), DVE partition-tree den (doubles DVE load).
"""

import os
import sys

sys.path.insert(0, "/opt/trn_rl_repo")

import numpy as np

import concourse.bacc as bacc
import concourse.tile as tile
from concourse import mybir

B, N, NH, H = 32, 512, 4, 32
IN_DIM, OUT_DIM = 128, 128
NCORES = 8
BS = B // NCORES  # batch elements per core
SQRT_DK = float(np.sqrt(H))
F32 = mybir.dt.float32
F16 = mybir.dt.float16
F8 = mybir.dt.float8e4
I16 = mybir.dt.int16
I8 = mybir.dt.int8
ADD = mybir.AluOpType.add
MULT = mybir.AluOpType.mult
EXP = mybir.ActivationFunctionType.Exp
LN = mybir.ActivationFunctionType.Ln
RELU = mybir.ActivationFunctionType.Relu

# FP8 mode: probabilities + v in fp8e4m3, av/den matmuls in DoubleRow
# (2 fp8 contraction elements per PE cell -> half the streaming cycles).
# Raw scores measure in [0, ~12] so exp(sc/sqrt_dk) is in [1, 8.2] -- the
# whole range sits in fp8e4m3's full-mantissa band, no flushing.
FP8 = os.environ.get("K_FP8", "0") == "1"

# Schraudolph constants: i{16,8} = (sc + CADD) * (M_VAL * mask); bitcast
# f16/f8e4 ~= exp(sc/sqrt_dk) (f16: 10-bit mantissa, 1024/oct; f8e4: 8/oct).
if FP8:
    M_VAL = float(np.float16(8.0 * np.log2(np.e) / SQRT_DK))  # ~2.041
    CADD = 8.0 * (7.0 - 0.043) / M_VAL
else:
    M_VAL = float(np.float16(1024.0 * np.log2(np.e) / SQRT_DK))  # 261.25
    C_SHIFT = 45.0
    CADD = (15.0 * 1024.0 - C_SHIFT) / M_VAL
BIAS_A = float(-np.log(M_VAL))  # ACT path: exp(sc/sqrt_dk - ln M) = e_true/M
PT_DT = F8 if FP8 else F16
PTI_DT = I8 if FP8 else I16
V_DT = F8 if FP8 else F16

# pair p -> (q group, k/v group); mask m{qg}{kg}; wo{qg}{kg}
PAIRS = [(0, 0), (1, 1), (0, 1), (1, 0)]
# pair -> (out group, concat row offset): inner pairs at rows 0:32, inter at 32:64
PAIR_DEST = [(0, 0), (1, 0), (0, 32), (1, 32)]

# Path per (chunk c, half hh) tile, indexed t = 2*c + hh (8 per pair).
# B: DVE Schraudolph; A: ACT exp + DVE mask-mul; C: ACT exp + GPSIMD mask-mul;
# D: ACT exp + DMA accumulate-multiply mask.
PATHS = list(os.environ.get("K_PATHS", "BCBABCBC"))
TAIL_DELAY = int(os.environ.get("K_TAIL_DELAY", "2"))  # in av chunk-pops
TAIL2 = int(os.environ.get("K_TAIL2", "2"))  # extra pops before an/wo phase
OUT_DELAY = int(os.environ.get("K_OUT_DELAY", "2"))  # extra pops before wf out
TLAG = int(os.environ.get("K_TLAG", "5"))  # score->av lag in chunks
ABLATE_DEN = os.environ.get("K_ABLATE_DEN", "0") == "1"  # timing diagnostic only
V_RELU_DVE = os.environ.get("K_VRELU_DVE", "0") == "1"  # v relu on DVE not ACT


def _pin_act_table(arch: str):
    """Narrow the cached act-table dict so every activation resolves to the
    natural_log_exp_and_others set (real HW table: Exp+Ln+Relu+Copy and
    more).  The greedy load-insertion pass picks the first table containing
    each function, which thrashes between exp_and_others and natural_log
    every softmax tail.  Only narrows the picker's view -- any table it does
    pick still genuinely contains the function on HW."""
    from concourse.hw_specs import get_activation_tables

    tabs = get_activation_tables(arch)
    if "natural_log_exp_and_others" not in tabs:
        return
    for k in list(tabs.keys()):
        if k != "natural_log_exp_and_others":
            tabs[k] = set()


def _emit_qkv(nc, pools, W, b, g, qt, kt, vt, xt_t):
    """QKV projection for (b, g): qt/kt/vt [128,512] f16 (v: [k, chunk*feat])."""
    xg = xt_t[:, 512 * g : 512 * (g + 1)]
    qp = pools["sc"].tile([128, N], F32, tag="sc", name="sc")
    nc.tensor.matmul(qp[:], W["wq"][g][:], xg, start=True, stop=True)
    nc.scalar.activation(qt[:], qp[:], RELU, bias=W["bq"][g][:])

    kp = pools["sc"].tile([128, N], F32, tag="sc", name="sc")
    nc.tensor.matmul(kp[:], W["wk"][g][:], xg, start=True, stop=True)
    nc.scalar.activation(kt[:], kp[:], RELU, bias=W["bk"][g][:])

    vp = pools["sc"].tile([128, N], F32, tag="sc", name="sc")
    # ones-row bias write opens the accumulation group (bias varies along free)
    nc.tensor.matmul(vp[:], W["onesrow"][:], W["bvr4"][g][:], start=True, stop=False)
    for c in range(4):
        nc.tensor.matmul(
            vp[:, 128 * c : 128 * (c + 1)],
            xt_t[:, 512 * g + 128 * c : 512 * g + 128 * (c + 1)],
            W["wv"][g][:],
            start=False,
            stop=False,
        )
    nc.tensor.matmul(vp[:], W["zrow16"][:], W["bvr4"][g][:], start=False, stop=True)
    if V_RELU_DVE:
        nc.vector.tensor_scalar_max(vt[:], vp[:], 0.0)
    else:
        nc.scalar.activation(vt[:], vp[:], RELU)


def _attn_helpers(nc, pools, W):
    """emit_front/emit_av/emit_tail closures; ctx = (mt_t, qt, kt, vt, cc)."""
    avt = {}
    pt4s = {}  # FP8: (b, p) -> [pt4_hh0, pt4_hh1], each [128, 4(c), 2(j), N] f8

    def emit_front(ctx, t):
        """One chunk: 4 adjacent score matmuls (row groups), then exp ops.

        Returns (t, pt_mm[4]) with pt_mm[h] the [128, N] prob tile slice
        (f16 mode), or (t, None) in FP8 mode (probs land in pt4s)."""
        mt_t, qt, kt, vt, cc = ctx
        b, p, c = t
        qg, kg = PAIRS[p]
        moff = (p * 4 + c) * 512
        mask_ap = mt_t[:, moff : moff + 512][:, None, :].broadcast_to([128, 2, N])
        scs = [
            pools["sc"].tile([128, 2, N], F32, tag="sc", name="sc") for _ in range(2)
        ]
        # 4 score matmuls back-to-back: disjoint 32-row groups run concurrently
        for hh in range(2):
            for j in range(2):
                h = 2 * hh + j
                nc.tensor.matmul(
                    scs[hh][:, j],
                    kt[kg][32 * h : 32 * (h + 1), 128 * c : 128 * (c + 1)],
                    qt[qg][32 * h : 32 * (h + 1), :],
                    start=True,
                    stop=True,
                    tile_position=(32 * h, 0),
                )
        if FP8 and c == 0:
            pt4s[(b, p)] = [
                pools["pt"].tile([128, 4, 2, N], F8, tag=f"pt4{hh}", name=f"pt4{hh}")
                for hh in range(2)
            ]
        pt_mm = [None] * 4
        for hh in range(2):
            sc = scs[hh]
            path = PATHS[2 * c + hh]
            if FP8:
                out_f8 = pt4s[(b, p)][hh][:, c]  # [128, 2, N] f8 slice
            if path == "B":
                if FP8:
                    nc.vector.scalar_tensor_tensor(
                        out_f8.bitcast(I8), sc[:], CADD, mask_ap, op0=ADD, op1=MULT
                    )
                else:
                    pti = pools["pt"].tile([128, 2, N], I16, tag="ptB", name="ptB")
                    nc.vector.scalar_tensor_tensor(
                        pti[:], sc[:], CADD, mask_ap, op0=ADD, op1=MULT
                    )
                    for j in range(2):
                        pt_mm[2 * hh + j] = pti[:, j].bitcast(F16)
            elif path == "D" and not FP8:
                # ACT exp straight into the prob tile, then a DMA
                # accumulate-multiply applies the mask in the DMA engines.
                pt = pools["pt"].tile([128, 2, N], F16, tag="ptA", name="ptA")
                nc.scalar.activation(
                    pt[:], sc[:], EXP, scale=1.0 / SQRT_DK, bias=W["biasA"][:]
                )
                nc.gpsimd.dma_start(out=pt[:], in_=mask_ap, accum_op=MULT)
                for j in range(2):
                    pt_mm[2 * hh + j] = pt[:, j]
            else:
                e = pools["e"].tile([128, 2, N], PT_DT, tag="e", name="e")
                nc.scalar.activation(
                    e[:], sc[:], EXP, scale=1.0 / SQRT_DK, bias=W["biasA"][:]
                )
                eng = nc.vector if path == "A" else nc.gpsimd
                if FP8:
                    eng.tensor_tensor(out_f8, e[:], mask_ap, op=MULT)
                else:
                    pt = pools["pt"].tile([128, 2, N], F16, tag="ptA", name="ptA")
                    eng.tensor_tensor(pt[:], e[:], mask_ap, op=MULT)
                    for j in range(2):
                        pt_mm[2 * hh + j] = pt[:, j]
        return (t, pt_mm)

    def emit_av(ctx, work):
        """One chunk: 4 adjacent av matmuls (col groups), then 4 den matmuls.

        start/stop land on the c==0 / c==3 matmul of each head slice.
        FP8: acts on odd chunks only -- one DoubleRow matmul covers the
        chunk pair u=c//2 with 256 fp8 contraction elements."""
        mt_t, qt, kt, vt, cc = ctx
        (b, p, c), pt_mm = work
        kg = PAIRS[p][1]
        key = (b, p)
        if key not in avt:
            av = pools["av"].tile([128, N], F32, tag="av", name="av")
            den = pools["den"].tile([128, N], F32, tag="den", name="den")
            avt[key] = (av, den)
        av, den = avt[key]
        if FP8:
            if c % 2 == 0:
                return
            u = c // 2
            first, last = u == 0, u == 1
            DR = mybir.MatmulPerfMode.DoubleRow
            for h in range(4):
                lhsT = (
                    vt[kg][:, 256 * u : 256 * (u + 1)]
                    .rearrange("p (j r) -> p j r", j=2)[:, :, 32 * h : 32 * (h + 1)]
                )
                nc.tensor.matmul(
                    av[32 * h : 32 * (h + 1), :],
                    lhsT,
                    pt4s[key][h // 2][:, 2 * u : 2 * u + 2, h % 2],
                    start=first,
                    stop=last,
                    tile_position=(0, 32 * h),
                    perf_mode=DR,
                    skip_group_check=True,
                )
            ones2 = W["ones64f8"][:].rearrange("p (j r) -> p j r", j=2)
            for h in range(4):
                nc.tensor.matmul(
                    den[32 * h : 32 * (h + 1), :],
                    ones2,
                    pt4s[key][h // 2][:, 2 * u : 2 * u + 2, h % 2],
                    start=first,
                    stop=last,
                    tile_position=(0, 32 * h),
                    perf_mode=DR,
                    skip_group_check=True,
                )
            if last:
                pt4s.pop(key)
            return
        first, last = c == 0, c == 3
        # skip_group_check: CoreSim's group-started bookkeeping mis-addresses
        # slices at nonzero partition offsets (tile-linear offset added to a
        # flat physical address); the functional pending-zero path is
        # per-memref and handles the per-head start/stop slices correctly.
        for h in range(4):
            nc.tensor.matmul(
                av[32 * h : 32 * (h + 1), :],
                vt[kg][:, 128 * c + 32 * h : 128 * c + 32 * (h + 1)],
                pt_mm[h],
                start=first,
                stop=last,
                tile_position=(0, 32 * h),
                skip_group_check=True,
            )
        if ABLATE_DEN:
            if first:
                for h in range(4):
                    nc.tensor.matmul(
                        den[32 * h : 32 * (h + 1), :],
                        W["ones32"][:],
                        pt_mm[h],
                        start=True,
                        stop=True,
                        tile_position=(0, 32 * h),
                        skip_group_check=True,
                    )
            return
        for h in range(4):
            nc.tensor.matmul(
                den[32 * h : 32 * (h + 1), :],
                W["ones32"][:],
                pt_mm[h],
                start=first,
                stop=last,
                tile_position=(0, 32 * h),
                skip_group_check=True,
            )

    def emit_tail1(ctx, b, p):
        """rden = 1/den via the custom-DVE fast reciprocal (1 op, ~51 ULP,
        ~5x faster than iterative divide; den is 100..1500 so no edge cases),
        then an = av * rden on the same DVE queue -- no cross-engine wait."""
        mt_t, qt, kt, vt, cc = ctx
        av, den = avt[(b, p)]
        rden = pools["rden"].tile([128, N], F32, tag="rden", name="rden")
        nc.vector.reciprocal_approx_fast(rden[:], den[:])
        an = pools["an"].tile([128, N], F16, tag="an", name="an")
        nc.vector.tensor_tensor(an[:], av[:], rden[:], op=MULT)
        avt[(b, p)] = an

    def emit_tail2(ctx, b, p):
        """wo projection + relu into the concat tile.  Emitted later than
        tail1 so the PE queue head doesn't sit waiting on the DVE an-mul
        (engine queues are strict FIFO)."""
        mt_t, qt, kt, vt, cc = ctx
        an = avt.pop((b, p))
        g, row = PAIR_DEST[p]
        wop = pools["sc"].tile([32, N], F32, tag="sc", name="sc")
        nc.tensor.matmul(wop[:], W["wo"][p][:], an[:], start=True, stop=True)
        nc.scalar.activation(cc[g][row : row + 32, :], wop[:], RELU, bias=W["bo"][p][:])

    return emit_front, emit_av, emit_tail1, emit_tail2


def _emit_out(nc, pools, W, b, g, cc):
    wfp = pools["sc"].tile([128, N], F32, tag="sc", name="sc")
    nc.tensor.matmul(wfp[:], W["wf"][g][:], cc[g][:], start=True, stop=True)
    ot = pools["ot"].tile([128, N], F16, tag="ot", name="ot")
    nc.scalar.copy(ot[:], wfp[:])
    nc.sync.dma_start(out=W["yt_ap"][b * 2 + g], in_=ot[:])


def build_nc(n_iters: int = 1):
    """Build + compile the per-core Bass module (body repeated n_iters times)."""
    import contextlib

    nc = bacc.Bacc("TRN2", target_bir_lowering=False, debug=False)
    _pin_act_table(nc.m.arch)

    xt = nc.dram_tensor("xt", [BS, 128, 2 * N], F16, kind="ExternalInput")
    mt = nc.dram_tensor("mt", [BS, 128, 16 * 512], F16, kind="ExternalInput")
    wqk = nc.dram_tensor("wqk", [2, 2, 128, 128], F16, kind="ExternalInput")
    wv = nc.dram_tensor("wv", [2, 128, 128], F16, kind="ExternalInput")
    bqk = nc.dram_tensor("bqk", [2, 2, 128, 1], F32, kind="ExternalInput")
    bvr4 = nc.dram_tensor("bvr4", [2, 1, 512], F16, kind="ExternalInput")
    wo = nc.dram_tensor("wo", [4, 128, 32], F16, kind="ExternalInput")
    bo = nc.dram_tensor("bo", [4, 32, 1], F32, kind="ExternalInput")
    wf = nc.dram_tensor("wf", [2, 65, 128], F16, kind="ExternalInput")
    onesrow = nc.dram_tensor("onesrow", [1, 128], F16, kind="ExternalInput")
    ones32 = nc.dram_tensor("ones32", [128, 32], F16, kind="ExternalInput")
    yt = nc.dram_tensor("yt", [BS * 2, 128, N], F16, kind="ExternalOutput")

    with tile.TileContext(nc) as tc, contextlib.ExitStack() as ctx:
        pools = {
            "consts": ctx.enter_context(tc.tile_pool(name="consts", bufs=1)),
            "xt": ctx.enter_context(tc.tile_pool(name="xt", bufs=2)),
            "persist": ctx.enter_context(tc.tile_pool(name="persist", bufs=1)),
            "mt": ctx.enter_context(tc.tile_pool(name="mt", bufs=2)),
            "e": ctx.enter_context(tc.tile_pool(name="e", bufs=6)),
            "pt": ctx.enter_context(tc.tile_pool(name="pt", bufs=3 if FP8 else 8)),
            "rden": ctx.enter_context(tc.tile_pool(name="rden", bufs=2)),
            "an": ctx.enter_context(tc.tile_pool(name="an", bufs=2)),
            "ot": ctx.enter_context(tc.tile_pool(name="ot", bufs=2)),
            "sc": ctx.enter_context(tc.tile_pool(name="sc", bufs=3, space="PSUM")),
            "av": ctx.enter_context(tc.tile_pool(name="av", bufs=1, space="PSUM")),
            "den": ctx.enter_context(tc.tile_pool(name="den", bufs=1, space="PSUM")),
        }
        cp = pools["consts"]
        W = {
            "yt_ap": yt.ap(),
            "wq": [cp.tile([128, 128], F16, tag=f"wq{g}", name=f"wq{g}") for g in range(2)],
            "wk": [cp.tile([128, 128], F16, tag=f"wk{g}", name=f"wk{g}") for g in range(2)],
            "wv": [cp.tile([128, 128], F16, tag=f"wv{g}", name=f"wv{g}") for g in range(2)],
            "bq": [cp.tile([128, 1], F32, tag=f"bq{g}", name=f"bq{g}") for g in range(2)],
            "bk": [cp.tile([128, 1], F32, tag=f"bk{g}", name=f"bk{g}") for g in range(2)],
            "bvr4": [cp.tile([1, 512], F16, tag=f"bvr4{g}", name=f"bvr4{g}") for g in range(2)],
            "zrow16": cp.tile([1, 128], F16, tag="zrow16", name="zrow16"),
            "wo": [cp.tile([128, 32], F16, tag=f"wo{p}", name=f"wo{p}") for p in range(4)],
            "bo": [cp.tile([32, 1], F32, tag=f"bo{p}", name=f"bo{p}") for p in range(4)],
            "wf": [cp.tile([65, 128], F16, tag=f"wf{g}", name=f"wf{g}") for g in range(2)],
            "onesrow": cp.tile([1, 128], F16, tag="onesrow", name="onesrow"),
            "ones32": cp.tile([128, 32], F16, tag="ones32", name="ones32"),
            "biasA": cp.tile([128, 1], F32, tag="biasA", name="biasA"),
        }
        if FP8:
            W["ones64f8"] = cp.tile([128, 64], F8, tag="ones64f8", name="ones64f8")
            nc.vector.memset(W["ones64f8"][:], 1.0)
        nc.vector.memset(W["biasA"][:], BIAS_A)
        for g in range(2):
            nc.sync.dma_start(out=W["wq"][g][:], in_=wqk.ap()[g, 0])
            nc.sync.dma_start(out=W["wk"][g][:], in_=wqk.ap()[g, 1])
            nc.sync.dma_start(out=W["wv"][g][:], in_=wv.ap()[g])
            nc.sync.dma_start(out=W["bq"][g][:], in_=bqk.ap()[g, 0])
            nc.sync.dma_start(out=W["bk"][g][:], in_=bqk.ap()[g, 1])
            nc.sync.dma_start(out=W["bvr4"][g][:], in_=bvr4.ap()[g])
            nc.sync.dma_start(out=W["wf"][g][:], in_=wf.ap()[g])
        for p in range(4):
            nc.sync.dma_start(out=W["wo"][p][:], in_=wo.ap()[p])
            nc.sync.dma_start(out=W["bo"][p][:], in_=bo.ap()[p])
        nc.sync.dma_start(out=W["onesrow"][:], in_=onesrow.ap())
        nc.vector.memset(W["zrow16"][:], 0.0)
        nc.sync.dma_start(out=W["ones32"][:], in_=ones32.ap())

        pp = pools["persist"]
        emit_front, emit_av, emit_tail1, emit_tail2 = _attn_helpers(nc, pools, W)
        # One software pipeline spans ALL n_iters: iteration boundaries would
        # otherwise drain the last pairs' tails serially (recip->an->wo->relu
        # ->wf->copy->dma with nothing overlapped) once per iteration.
        # Elements are a global index e; tile tags fold to e % BS (bounded
        # SBUF -- the bufs=1 persist ring makes reuse a WAR dep ~3 elems old).
        n_elems = n_iters * BS

        def _qkv_tiles(eb):
            q = [pp.tile([128, N], F16, tag=f"qt{eb}{g}", name=f"qt{eb}{g}") for g in range(2)]
            k = [pp.tile([128, N], F16, tag=f"kt{eb}{g}", name=f"kt{eb}{g}") for g in range(2)]
            v = [pp.tile([128, N], V_DT, tag=f"vt{eb}{g}", name=f"vt{eb}{g}") for g in range(2)]
            return q, k, v

        ctxs = {}

        def prep_b(e):
            if e >= n_elems or e in ctxs:
                return
            eb = e % BS
            xt_t = pools["xt"].tile([128, 2 * N], F16, tag="xt", name="xt")
            nc.sync.dma_start(out=xt_t[:], in_=xt.ap()[eb])
            mt_t = pools["mt"].tile([128, 16 * 512], F16, tag="mt", name="mt")
            nc.sync.dma_start(out=mt_t[:], in_=mt.ap()[eb])
            qt, kt, vt = _qkv_tiles(eb)
            for g in range(2):
                _emit_qkv(nc, pools, W, eb, g, qt[g], kt[g], vt[g], xt_t)
            cc = [
                pp.tile([65, N], F16, tag=f"cc{eb}{g}", name=f"cc{eb}{g}")
                for g in range(2)
            ]
            for g in range(2):
                nc.gpsimd.memset(cc[g][64:65, :], 1.0)
            ctxs[e] = (mt_t, qt, kt, vt, cc)

        prep_b(0)
        chunks = [
            (e, p, c) for e in range(n_elems) for p in range(4) for c in range(4)
        ]
        pending = []
        done_av = {}
        actions = []  # (due_pop, seq, fn) -- delayed cross-engine tails
        seq_ctr = [0]
        npops = [0]

        def schedule(due, fn):
            actions.append((due, seq_ctr[0], fn))
            seq_ctr[0] += 1

        def flush_actions(force=False):
            actions.sort(key=lambda a: (a[0], a[1]))
            while actions and (force or actions[0][0] <= npops[0]):
                actions.pop(0)[2]()

        def pop_one():
            w = pending.pop(0)
            (we, wp, _), _ = w
            emit_av(ctxs[we], w)
            npops[0] += 1
            done_av[(we, wp)] = done_av.get((we, wp), 0) + 1
            if done_av[(we, wp)] == 4:
                n = npops[0]
                schedule(n + TAIL_DELAY, lambda e=we, p=wp: emit_tail1(ctxs[e], e, p))
                schedule(n + TAIL_DELAY + TAIL2, lambda e=we, p=wp: emit_tail2(ctxs[e], e, p))
                if wp == 3:
                    def outs(e=we):
                        for g in range(2):
                            _emit_out(nc, pools, W, e % BS, g, ctxs[e][4])
                        del ctxs[e]
                    schedule(n + TAIL_DELAY + TAIL2 + OUT_DELAY, outs)
            flush_actions()

        # Prep the next element at pair PREP_AT (not pair 0): the element
        # boundary already clusters the previous element's tail/out matmuls
        # (wo/wf, full-array, serializing); spreading the next element's 14
        # qkv matmuls to mid-element keeps the packed-matmul stream dense.
        # 12 chunks of lead still covers the 5.6us mask DMA.
        prep_at = int(os.environ.get("K_PREP_AT", "0"))
        for t in chunks:
            e = t[0]
            if t[1] == prep_at and t[2] == 0:
                prep_b(e + 1)  # staggered qkv for the next element / iteration
            pending.append(emit_front(ctxs[e], t))
            if len(pending) > TLAG:
                pop_one()
        while pending:
            pop_one()
        flush_actions(force=True)

    nc.compile()
    return nc


def prep_weights(inp):
    """Host-side packing of the (core-replicated) weight tensors."""
    f = np.asarray
    W = {}
    W["wqk"] = np.stack(
        [
            np.stack([f(inp["wq0"]), f(inp["wk0"])]),
            np.stack([f(inp["wq1"]), f(inp["wk1"])]),
        ]
    ).astype(np.float16)
    W["wv"] = np.stack([f(inp["wv0"]), f(inp["wv1"])]).astype(np.float16)
    W["bqk"] = np.stack(
        [
            np.stack([f(inp["bq0"]).reshape(128, 1), f(inp["bk0"]).reshape(128, 1)]),
            np.stack([f(inp["bq1"]).reshape(128, 1), f(inp["bk1"]).reshape(128, 1)]),
        ]
    ).astype(np.float32)
    W["bvr4"] = np.stack(
        [np.tile(f(inp["bv0"]), 4).reshape(1, 512), np.tile(f(inp["bv1"]), 4).reshape(1, 512)]
    ).astype(np.float16)
    W["wo"] = np.stack(
        [f(inp["wo00"]), f(inp["wo11"]), f(inp["wo01"]), f(inp["wo10"])]
    ).astype(np.float16)
    W["bo"] = np.stack(
        [
            f(inp["bo00"]).reshape(32, 1),
            f(inp["bo11"]).reshape(32, 1),
            f(inp["bo01"]).reshape(32, 1),
            f(inp["bo10"]).reshape(32, 1),
        ]
    ).astype(np.float32)
    wf_stack = []
    for g in range(2):
        wfg = np.concatenate(
            [f(inp[f"wf{g}"]), f(inp[f"bf{g}"]).reshape(1, 128)], axis=0
        )  # [65, 128]
        wf_stack.append(wfg)
    W["wf"] = np.stack(wf_stack).astype(np.float16)
    W["onesrow"] = np.ones((1, 128), np.float16)
    W["ones32"] = np.ones((128, 32), np.float16)
    return W


def prep_core_inputs(inp, W):
    """Build the 8 per-core in_maps (shards batch over cores)."""
    x = [np.asarray(inp["x0"], np.float32), np.asarray(inp["x1"], np.float32)]
    masks = [
        np.asarray(inp["m00"]),
        np.asarray(inp["m11"]),
        np.asarray(inp["m01"]),
        np.asarray(inp["m10"]),
    ]
    in_maps = []
    for ci in range(NCORES):
        xt = np.empty((BS, 128, 2 * N), np.float16)
        mtv = np.empty((BS, 128, 16 * 512), np.float16)
        for b in range(BS):
            gb = ci * BS + b
            for g in range(2):
                xt[b, :, 512 * g : 512 * (g + 1)] = x[g][gb].T
            for p in range(4):
                mT = masks[p][gb].T.astype(np.float16) * np.float16(M_VAL)  # [k, q]
                ch = mT.reshape(4, 128, N)  # chunk c = k rows 128c..
                mtv[b, :, (p * 4) * 512 : (p * 4 + 4) * 512] = (
                    ch.transpose(1, 0, 2).reshape(128, 4 * N)
                )
        m = {"xt": xt, "mt": mtv}
        m.update(W)
        in_maps.append(m)
    return in_maps


def postprocess(results):
    """Gather per-core yt [8,128,512] -> (out0, out1) full arrays."""
    out0 = np.empty((B, N, OUT_DIM), np.float32)
    out1 = np.empty((B, N, OUT_DIM), np.float32)
    for ci in range(NCORES):
        yt = results[ci]["yt"]
        for b in range(BS):
            gb = ci * BS + b
            out0[gb] = yt[b * 2 + 0].T
            out1[gb] = yt[b * 2 + 1].T
    return out0, out1


_NC_CACHE = {}


def get_nc(n_iters: int = 1):
    if n_iters not in _NC_CACHE:
        _NC_CACHE[n_iters] = build_nc(n_iters)
    return _NC_CACHE[n_iters]


def kernel(**inputs):
    from concourse import bass_utils

    nc = get_nc(1)
    W = prep_weights(inputs)
    in_maps = prep_core_inputs(inputs, W)
    res = bass_utils.run_bass_kernel_spmd(
        nc, in_maps, core_ids=list(range(NCORES)), trace=False
    )
    return postprocess(res.results)
